# revision 4
# baseline (speedup 1.0000x reference)
"""Trainium2 Bass kernel for expected-calibration-error (ECE) over [N,C] logits.

Contract: kernel(logits, targets) -> np.float32 scalar (shape ()), matching

    probs = softmax(logits); conf = max(probs); pred = argmax(probs)
    acc = (pred == targets); bins of width 1/10 over (k/10, (k+1)/10]
    ECE = sum_k |avg_conf_k - avg_acc_k| * count_k / N

Strategy (data-parallel over 8 NeuronCores, rows sharded; v2):
  * Host converts logits to fp16 row layout [128, TW*C] and gathers the
    target-class logit per row (tl), as in v1.
  * Per-row MAX stays a DVE fp16 TT fold chain (128->64->32->16->8 at the
    2x perf mode, then a short tensor_reduce) -- ~70 ns/col, the cheapest
    reduction this ISA offers.
  * Per-row SUMEXP moves OFF the DVE entirely:
      - each 128x128 block of y is transposed SBUF->SBUF by the XBAR
        transpose DMA (14 ns per 16x128 tile, rides otherwise-idle DMA
        queues), giving y_T blocks with classes on partitions;
      - exp is computed elementwise on y_T, split between the scalar
        engine (true Exp, biased by ln K) and the DVE (Schraudolph
        fast-exp: one tensor_scalar mult+add at the 4x perf mode writing
        int16 bits that reinterpret as fp16 ~ K*exp(y); verified
        bit-exact on HW, K = E[(1+f)/2^f] = 1.0406845);
      - the tensor engine reduces each E_T block with a stationary-weights
        matmul (lhsT = E_T block, rhs = ones[128,1]) writing S' as a
        [128,1] PSUM column -- the output lands PSUM[p, j], exactly
        aligned with the row layout (row = (p, j)), so no realignment or
        PSUM evacuation is needed.
  * conf' = exp(m) * recip(S') = conf / K; bin thresholds are pre-scaled
    by 1/K on the host and sum_conf is multiplied back by K in finalize,
    so the fast-exp's mean inflation cancels exactly. Residual per-row
    noise (+-0.26%, validated on the real data: ECE rel err 1.9e-4) is
    far under the 2e-2 gate.
  * acc = (tl == rowmax), exact in fp16.
  * Bin masks: DVE is_gt broadcast ({0,1} masks, 2x mode); histogram
    triples accumulate in PSUM via one tiny PE matmul per 128-row column
    (28 ns each measured); host differences adjacent cumulative columns.
"""

import numpy as np

# Problem constants (hardcoded per harness contract).
N = 262144
C = 128
P = 128          # SBUF partitions
NB = 10          # calibration bins
NCORES = 8
ROWS_PER_CORE = N // NCORES          # 32768
TW = ROWS_PER_CORE // P              # 256 row-blocks (columns) per core

# fast-exp constants: n = round(y*1024/ln2 + 15360) bit-viewed as fp16
FS = 1477.3197
FO = 15360.0
K_CORR = 1.0406845  # E[(1+f)/2^f], f ~ U(0,1); mean inflation of fast-exp

# chunk sizes (columns of 128 rows); must sum to TW
CHUNKS = (16, 16, 32, 32, 32, 32, 32, 32, 32)
# exp engine per chunk: 'dve' fast-exp (4x TSP) or 'act' true exp
EXP_ENG = ("dve", "act", "dve", "act", "dve", "act", "dve", "act", "dve")
GW = 64          # phase-2 group width (columns)

_CACHE = {}


def build(chunks=CHUNKS, exp_eng=EXP_ENG, gw=GW):
    """Build the Bass module. Returns nc."""
    import concourse.bacc as bacc
    import concourse.tile as tile
    from concourse import mybir

    f32 = mybir.dt.float32
    f16 = mybir.dt.float16
    i16 = mybir.dt.int16
    Alu = mybir.AluOpType
    Act = mybir.ActivationFunctionType
    X = mybir.AxisListType.X

    assert sum(chunks) == TW
    assert TW % gw == 0
    ngroups = TW // gw
    lnK = float(np.log(K_CORR))

    nc = bacc.Bacc(trn_type="TRN2")

    y_d = nc.dram_tensor("y", [P, TW * C], f16, kind="ExternalInput")
    tl_d = nc.dram_tensor("tl", [P, TW], f16, kind="ExternalInput")
    thr_d = nc.dram_tensor("thr", [1, NB + 1], f16, kind="ExternalInput")
    out_d = nc.dram_tensor("gstats", [3, NB + 1], f32, kind="ExternalOutput")

    with tile.TileContext(nc) as tc:
        with (
            tc.tile_pool(name="yT", bufs=3) as yT_pool,
            tc.tile_pool(name="eT", bufs=3) as eT_pool,
            tc.tile_pool(name="fold", bufs=3) as f_pool,
            tc.tile_pool(name="grp", bufs=3) as grp_pool,
            tc.tile_pool(name="single", bufs=1) as single,
            tc.tile_pool(name="psum", bufs=1, space="PSUM") as psum_pool,
        ):
            y_rl = single.tile([P, TW * C], f16)   # full y resident (8.4MB)
            tl_all = single.tile([P, TW], f16)
            nc.sync.dma_start(out=tl_all[:], in_=tl_d[:])
            thr_sb = single.tile([P, NB + 1], f16)
            nc.sync.dma_start(out=thr_sb[:], in_=thr_d[:].partition_broadcast(P))

            m_all = single.tile([P, TW], f16)
            rhs3 = single.tile([P, 3, TW], f16)
            nc.gpsimd.memset(rhs3[:, 0, :], 1.0)
            ones_w = single.tile([P, 1], f16)
            nc.gpsimd.memset(ones_w[:], 1.0)
            lnK_ap = single.tile([P, 1], f32)
            nc.gpsimd.memset(lnK_ap[:], lnK)

            S_ps = psum_pool.tile([P, TW], f32)
            pstats = psum_pool.tile([3, NB + 1], f32)

            def do_chunk(o, w, eng):
                # 1) load chunk of y
                nc.sync.dma_start(
                    out=y_rl[:, o * C : (o + w) * C],
                    in_=y_d[:, o * C : (o + w) * C],
                )
                y3 = y_rl[:, o * C : (o + w) * C].rearrange(
                    "p (t c) -> p t c", c=C
                )
                # 2) DVE max fold chain at 2x, tail reduce
                cur, hw_ = y3, C
                while hw_ > 8:
                    h = hw_ // 2
                    M = f_pool.tile([P, 32 * 64], f16, name="Mv")
                    M3 = M[:, : w * h].rearrange("p (t c) -> p t c", c=h)
                    nc.vector.tensor_tensor(
                        out=M3, in0=cur[:, :, 0:h], in1=cur[:, :, h:hw_],
                        op=Alu.max,
                    )
                    cur, hw_ = M3, h
                nc.vector.tensor_reduce(
                    out=m_all[:, o : o + w], in_=cur, axis=X, op=Alu.max
                )
                # 3) xbar-transpose each 128x128 block into y_T
                yT = yT_pool.tile([P, 32 * 128], f16, name="yT")
                for j in range(w):
                    nc.sync.dma_start_transpose(
                        out=yT[:, j * 128 : (j + 1) * 128],
                        in_=y_rl[:, (o + j) * C : (o + j + 1) * C],
                    )
                # 4) exp on the transposed chunk
                eT = eT_pool.tile([P, 32 * 128], f16, name="eT")
                if eng == "act":
                    # true exp scaled by K: exp(y + lnK)
                    nc.scalar.activation(
                        out=eT[:, : w * 128], in_=yT[:, : w * 128],
                        func=Act.Exp, bias=lnK_ap[:],
                    )
                else:
                    # Schraudolph fast-exp ~ K*exp(y) in fp16 bits, 4x TSP
                    nc.vector.tensor_scalar(
                        out=eT[:, : w * 128].bitcast(i16),
                        in0=yT[:, : w * 128],
                        scalar1=FS, scalar2=FO, op0=Alu.mult, op1=Alu.add,
                    )
                # 5) PE stationary-weights column sums: S'[p, o+j]
                for j in range(w):
                    nc.tensor.matmul(
                        S_ps[:, o + j : o + j + 1],
                        eT[:, j * 128 : (j + 1) * 128],
                        ones_w[:],
                        start=True, stop=True,
                        skip_group_check=True,
                    )

            def phase2(grp):
                c0, c1 = grp * gw, (grp + 1) * gw
                e_m = grp_pool.tile([P, gw], f32, name="em")
                nc.scalar.activation(out=e_m[:], in_=m_all[:, c0:c1],
                                     func=Act.Exp)
                rs = grp_pool.tile([P, gw], f32, name="rs")
                nc.vector.reciprocal_approx_fast(out=rs[:], in_=S_ps[:, c0:c1])
                nc.vector.tensor_tensor(
                    out=rhs3[:, 1, c0:c1], in0=e_m[:], in1=rs[:], op=Alu.mult
                )
                # acc: target logit attains the row max (exact in fp16)
                nc.vector.tensor_tensor(
                    out=rhs3[:, 2, c0:c1], in0=m_all[:, c0:c1],
                    in1=tl_all[:, c0:c1], op=Alu.is_equal,
                )
                # {0,1} cumulative bin masks, DVE is_gt broadcast at 2x
                g = grp_pool.tile([P, gw, NB + 1], f16, name="gv")
                cb = rhs3[:, 1, c0:c1].unsqueeze(2).broadcast_to(
                    [P, gw, NB + 1]
                )
                tb = thr_sb[:].unsqueeze(1).broadcast_to([P, gw, NB + 1])
                nc.vector.tensor_tensor(out=g[:], in0=cb, in1=tb, op=Alu.is_gt)
                # per-column cumulative histogram triples on PE
                for j in range(gw):
                    nc.tensor.matmul(
                        pstats[:],
                        rhs3[:, :, c0 + j],
                        g[:, j, :],
                        start=(grp == 0 and j == 0),
                        stop=(grp == ngroups - 1 and j == gw - 1),
                        skip_group_check=True,
                    )

            pending = 0
            done = 0
            for k, w in enumerate(chunks):
                do_chunk(done, w, exp_eng[k])
                done += w
                while pending < ngroups and done >= (pending + 1) * gw + 16:
                    phase2(pending)
                    pending += 1
            while pending < ngroups:
                phase2(pending)
                pending += 1

            stats_sb = single.tile([3, NB + 1], f32)
            nc.vector.tensor_copy(out=stats_sb[:], in_=pstats[:])
            nc.sync.dma_start(out=out_d[:], in_=stats_sb[:])

    nc.compile()
    return nc


def prep_inputs(logits, targets, ncores=NCORES):
    """Convert + shard host inputs. Returns list of per-core in_maps."""
    l = np.asarray(logits, dtype=np.float32)
    tg = np.asarray(targets).astype(np.int64)
    n = l.shape[0]

    y16 = l.astype(np.float16)
    tl16 = y16[np.arange(n), tg]
    thr = (np.arange(NB + 1, dtype=np.float64) / NB / K_CORR).reshape(
        1, NB + 1
    ).astype(np.float16)

    rpc = n // ncores
    in_maps = []
    for k in range(ncores):
        yk = y16[k * rpc : (k + 1) * rpc].reshape(P, TW * C)
        tlk = tl16[k * rpc : (k + 1) * rpc].reshape(P, TW)
        in_maps.append(
            {"y": np.ascontiguousarray(yk), "tl": np.ascontiguousarray(tlk),
             "thr": thr}
        )
    return in_maps


def finalize(gstats_list, n=N):
    """Combine per-core cumulative [3, 11] stats into the ECE scalar.

    G[:, k] = (count, sum_conf', sum_acc) over rows with conf' > T_k/K.
    """
    G = np.zeros((3, NB + 1), dtype=np.float64)
    for gs in gstats_list:
        G += gs.astype(np.float64)
    per = G[:, 0:NB] - G[:, 1 : NB + 1]
    counts, sum_conf, sum_acc = per[0], per[1] * K_CORR, per[2]
    safe = np.maximum(counts, 1.0)
    avg_conf = sum_conf / safe
    avg_acc = sum_acc / safe
    prop = counts / float(n)
    ece = np.where(counts > 0, np.abs(avg_conf - avg_acc) * prop, 0.0).sum()
    return np.array(ece, dtype=np.float32)


LAST_RESULTS = None  # BassKernelResults of the most recent kernel() call


def kernel(logits, targets):
    global LAST_RESULTS
    from concourse.bass_utils import run_bass_kernel_spmd

    key = (CHUNKS, EXP_ENG, GW)
    if key not in _CACHE:
        _CACHE[key] = build(CHUNKS, EXP_ENG, GW)
    nc = _CACHE[key]

    in_maps = prep_inputs(logits, targets)
    res = run_bass_kernel_spmd(nc, in_maps, core_ids=list(range(NCORES)))
    LAST_RESULTS = res
    return finalize([r["gstats"] for r in res.results])


# revision 5
# speedup vs baseline: 4.2833x; 4.2833x over previous
"""Trainium2 Bass kernel for expected-calibration-error (ECE) over [N,C] logits.

Contract: kernel(logits, targets) -> np.float32 scalar (shape ()), matching

    probs = softmax(logits); conf = max(probs); pred = argmax(probs)
    acc = (pred == targets); bins of width 1/10 over (k/10, (k+1)/10]
    ECE = sum_k |avg_conf_k - avg_acc_k| * count_k / N

Strategy (data-parallel over 8 NeuronCores, rows sharded; v2):
  * Host converts logits to fp16 row layout [128, TW*C] and gathers the
    target-class logit per row (tl), as in v1.
  * Per-row MAX stays a DVE fp16 TT fold chain (128->64->32->16->8 at the
    2x perf mode, then a short tensor_reduce) -- ~70 ns/col, the cheapest
    reduction this ISA offers.
  * Per-row SUMEXP moves OFF the DVE entirely:
      - each 128x128 block of y is transposed SBUF->SBUF by the XBAR
        transpose DMA (14 ns per 16x128 tile, rides otherwise-idle DMA
        queues), giving y_T blocks with classes on partitions;
      - exp is computed elementwise on y_T, split between the scalar
        engine (true Exp, biased by ln K) and the DVE (Schraudolph
        fast-exp: one tensor_scalar mult+add at the 4x perf mode writing
        int16 bits that reinterpret as fp16 ~ K*exp(y); verified
        bit-exact on HW, K = E[(1+f)/2^f] = 1.0406845);
      - the tensor engine reduces each E_T block with a stationary-weights
        matmul (lhsT = E_T block, rhs = ones[128,1]) writing S' as a
        [128,1] PSUM column -- the output lands PSUM[p, j], exactly
        aligned with the row layout (row = (p, j)), so no realignment or
        PSUM evacuation is needed.
  * conf' = exp(m) * recip(S') = conf / K; bin thresholds are pre-scaled
    by 1/K on the host and sum_conf is multiplied back by K in finalize,
    so the fast-exp's mean inflation cancels exactly. Residual per-row
    noise (+-0.26%, validated on the real data: ECE rel err 1.9e-4) is
    far under the 2e-2 gate.
  * acc = (tl == rowmax), exact in fp16.
  * Bin masks: DVE is_gt broadcast ({0,1} masks, 2x mode); histogram
    triples accumulate in PSUM via one tiny PE matmul per 128-row column
    (28 ns each measured); host differences adjacent cumulative columns.
"""

import numpy as np

# Problem constants (hardcoded per harness contract).
N = 262144
C = 128
P = 128          # SBUF partitions
NB = 10          # calibration bins
NCORES = 8
ROWS_PER_CORE = N // NCORES          # 32768
TW = ROWS_PER_CORE // P              # 256 row-blocks (columns) per core

# fast-exp constants: n = round(y*1024/ln2 + 15360) bit-viewed as fp16
FS = 1477.3197
FO = 15360.0
K_CORR = 1.0406845  # E[(1+f)/2^f], f ~ U(0,1); mean inflation of fast-exp

# chunk sizes (columns of 128 rows); must sum to TW
CHUNKS = (16, 16, 32, 32, 32, 32, 32, 32, 32)
# exp engine per chunk: 'dve' fast-exp (4x TSP) or 'act' true exp
EXP_ENG = ("dve", "act", "dve", "act", "dve", "act", "dve", "act", "dve")
GW = 64          # phase-2 group width (columns)

_CACHE = {}


def build(chunks=CHUNKS, exp_eng=EXP_ENG, gw=GW):
    """Build the Bass module. Returns nc."""
    import concourse.bacc as bacc
    import concourse.tile as tile
    from concourse import mybir

    f32 = mybir.dt.float32
    f16 = mybir.dt.float16
    i16 = mybir.dt.int16
    Alu = mybir.AluOpType
    Act = mybir.ActivationFunctionType
    X = mybir.AxisListType.X

    assert sum(chunks) == TW
    assert TW % gw == 0
    ngroups = TW // gw
    lnK = float(np.log(K_CORR))

    nc = bacc.Bacc(trn_type="TRN2")

    y_d = nc.dram_tensor("y", [P, TW * C], f16, kind="ExternalInput")
    tl_d = nc.dram_tensor("tl", [P, TW], f16, kind="ExternalInput")
    thr_d = nc.dram_tensor("thr", [1, NB + 1], f16, kind="ExternalInput")
    out_d = nc.dram_tensor("gstats", [3, NB + 1], f32, kind="ExternalOutput")

    with tile.TileContext(nc) as tc:
        with (
            tc.tile_pool(name="yT", bufs=3) as yT_pool,
            tc.tile_pool(name="eT", bufs=3) as eT_pool,
            tc.tile_pool(name="fold", bufs=3) as f_pool,
            tc.tile_pool(name="grp", bufs=3) as grp_pool,
            tc.tile_pool(name="single", bufs=1) as single,
            tc.tile_pool(name="psum", bufs=1, space="PSUM") as psum_pool,
        ):
            y_rl = single.tile([P, TW * C], f16)   # full y resident (8.4MB)
            tl_all = single.tile([P, TW], f16)
            nc.sync.dma_start(out=tl_all[:], in_=tl_d[:])
            thr_sb = single.tile([P, NB + 1], f16)
            nc.sync.dma_start(out=thr_sb[:], in_=thr_d[:].partition_broadcast(P))

            m_all = single.tile([P, TW], f16)
            rhs3 = single.tile([P, 3, TW], f16)
            nc.gpsimd.memset(rhs3[:, 0, :], 1.0)
            ones_w = single.tile([P, 1], f16)
            nc.gpsimd.memset(ones_w[:], 1.0)
            lnK_ap = single.tile([P, 1], f32)
            nc.gpsimd.memset(lnK_ap[:], lnK)

            S_ps = psum_pool.tile([P, TW], f32)
            pstats = psum_pool.tile([3, NB + 1], f32)

            def do_chunk(o, w, eng):
                # 1) load chunk of y
                nc.sync.dma_start(
                    out=y_rl[:, o * C : (o + w) * C],
                    in_=y_d[:, o * C : (o + w) * C],
                )
                y3 = y_rl[:, o * C : (o + w) * C].rearrange(
                    "p (t c) -> p t c", c=C
                )
                # 2) DVE max fold chain at 2x, tail reduce
                cur, hw_ = y3, C
                while hw_ > 8:
                    h = hw_ // 2
                    M = f_pool.tile([P, 32 * 64], f16, name="Mv")
                    M3 = M[:, : w * h].rearrange("p (t c) -> p t c", c=h)
                    nc.vector.tensor_tensor(
                        out=M3, in0=cur[:, :, 0:h], in1=cur[:, :, h:hw_],
                        op=Alu.max,
                    )
                    cur, hw_ = M3, h
                nc.vector.tensor_reduce(
                    out=m_all[:, o : o + w], in_=cur, axis=X, op=Alu.max
                )
                # 3) xbar-transpose the whole chunk blockwise in ONE DMA:
                #    out[c, j*128+p] = in[p, j*128+c] (verified on HW)
                yT = yT_pool.tile([P, 32 * 128], f16, name="yT")
                nc.sync.dma_start_transpose(
                    out=yT[:, : w * 128].rearrange("p (j q) -> p j q", q=128),
                    in_=y_rl[:, o * C : (o + w) * C],
                )
                # 4) exp on the transposed chunk
                eT = eT_pool.tile([P, 32 * 128], f16, name="eT")
                if eng == "act":
                    # true exp scaled by K: exp(y + lnK)
                    nc.scalar.activation(
                        out=eT[:, : w * 128], in_=yT[:, : w * 128],
                        func=Act.Exp, bias=lnK_ap[:],
                    )
                else:
                    # Schraudolph fast-exp ~ K*exp(y) in fp16 bits, 4x TSP
                    nc.vector.tensor_scalar(
                        out=eT[:, : w * 128].bitcast(i16),
                        in0=yT[:, : w * 128],
                        scalar1=FS, scalar2=FO, op0=Alu.mult, op1=Alu.add,
                    )
                # 5) PE stationary-weights column sums: S'[p, o+j]
                for j in range(w):
                    nc.tensor.matmul(
                        S_ps[:, o + j : o + j + 1],
                        eT[:, j * 128 : (j + 1) * 128],
                        ones_w[:],
                        start=True, stop=True,
                        skip_group_check=True,
                    )

            def phase2(grp):
                c0, c1 = grp * gw, (grp + 1) * gw
                e_m = grp_pool.tile([P, gw], f32, name="em")
                nc.scalar.activation(out=e_m[:], in_=m_all[:, c0:c1],
                                     func=Act.Exp)
                rs = grp_pool.tile([P, gw], f32, name="rs")
                nc.vector.reciprocal_approx_fast(out=rs[:], in_=S_ps[:, c0:c1])
                nc.vector.tensor_tensor(
                    out=rhs3[:, 1, c0:c1], in0=e_m[:], in1=rs[:], op=Alu.mult
                )
                # acc: target logit attains the row max (exact in fp16)
                nc.vector.tensor_tensor(
                    out=rhs3[:, 2, c0:c1], in0=m_all[:, c0:c1],
                    in1=tl_all[:, c0:c1], op=Alu.is_equal,
                )
                # {0,1} cumulative bin masks, DVE is_gt broadcast at 2x
                g = grp_pool.tile([P, gw, NB + 1], f16, name="gv")
                cb = rhs3[:, 1, c0:c1].unsqueeze(2).broadcast_to(
                    [P, gw, NB + 1]
                )
                tb = thr_sb[:].unsqueeze(1).broadcast_to([P, gw, NB + 1])
                nc.vector.tensor_tensor(out=g[:], in0=cb, in1=tb, op=Alu.is_gt)
                # per-column cumulative histogram triples on PE
                for j in range(gw):
                    nc.tensor.matmul(
                        pstats[:],
                        rhs3[:, :, c0 + j],
                        g[:, j, :],
                        start=(grp == 0 and j == 0),
                        stop=(grp == ngroups - 1 and j == gw - 1),
                        skip_group_check=True,
                    )

            pending = 0
            done = 0
            for k, w in enumerate(chunks):
                do_chunk(done, w, exp_eng[k])
                done += w
                while pending < ngroups and done >= (pending + 1) * gw + 16:
                    phase2(pending)
                    pending += 1
            while pending < ngroups:
                phase2(pending)
                pending += 1

            stats_sb = single.tile([3, NB + 1], f32)
            nc.vector.tensor_copy(out=stats_sb[:], in_=pstats[:])
            nc.sync.dma_start(out=out_d[:], in_=stats_sb[:])

    nc.compile()
    return nc


def prep_inputs(logits, targets, ncores=NCORES):
    """Convert + shard host inputs. Returns list of per-core in_maps."""
    l = np.asarray(logits, dtype=np.float32)
    tg = np.asarray(targets).astype(np.int64)
    n = l.shape[0]

    y16 = l.astype(np.float16)
    tl16 = y16[np.arange(n), tg]
    thr = (np.arange(NB + 1, dtype=np.float64) / NB / K_CORR).reshape(
        1, NB + 1
    ).astype(np.float16)

    rpc = n // ncores
    in_maps = []
    for k in range(ncores):
        yk = y16[k * rpc : (k + 1) * rpc].reshape(P, TW * C)
        tlk = tl16[k * rpc : (k + 1) * rpc].reshape(P, TW)
        in_maps.append(
            {"y": np.ascontiguousarray(yk), "tl": np.ascontiguousarray(tlk),
             "thr": thr}
        )
    return in_maps


def finalize(gstats_list, n=N):
    """Combine per-core cumulative [3, 11] stats into the ECE scalar.

    G[:, k] = (count, sum_conf', sum_acc) over rows with conf' > T_k/K.
    """
    G = np.zeros((3, NB + 1), dtype=np.float64)
    for gs in gstats_list:
        G += gs.astype(np.float64)
    per = G[:, 0:NB] - G[:, 1 : NB + 1]
    counts, sum_conf, sum_acc = per[0], per[1] * K_CORR, per[2]
    safe = np.maximum(counts, 1.0)
    avg_conf = sum_conf / safe
    avg_acc = sum_acc / safe
    prop = counts / float(n)
    ece = np.where(counts > 0, np.abs(avg_conf - avg_acc) * prop, 0.0).sum()
    return np.array(ece, dtype=np.float32)


LAST_RESULTS = None  # BassKernelResults of the most recent kernel() call


def kernel(logits, targets):
    global LAST_RESULTS
    from concourse.bass_utils import run_bass_kernel_spmd

    key = (CHUNKS, EXP_ENG, GW)
    if key not in _CACHE:
        _CACHE[key] = build(CHUNKS, EXP_ENG, GW)
    nc = _CACHE[key]

    in_maps = prep_inputs(logits, targets)
    res = run_bass_kernel_spmd(nc, in_maps, core_ids=list(range(NCORES)))
    LAST_RESULTS = res
    return finalize([r["gstats"] for r in res.results])
